# revision 10
# baseline (speedup 1.0000x reference)
"""Multi-head attention (B=4, S=2048, D=1024, H=16) on 8 TRN2 NeuronCores.

Load-balanced sharding: every core handles heads {2c, 2c+1} of ALL four
batches. Per-batch attention depth kt_b = ceil(valid_len_b / 128) is baked
into the program (identical on every core -> SPMD-safe), so the per-core
work is Sum_b kt_b head-key-tiles regardless of how skewed valid_lens are.
Keys/values are truncated to kt_b*128 rows (masked keys contribute exp=0).

Per-core dataflow (all matmuls bf16 on the PE array):
  QT[b][d',q]   = (Xq[b] Wq_slice)^T      d' = 128 dims of 2 heads
  KT[b][d',k]   = (Xk[b,:Kb] Wk_slice)^T
  V[b][k,s,65]  = Xv[b,:Kb] Wv_slice      (ones column -> softmax denom)
  per (b, q-half, key tile): scores pair (64-dim contraction, heads at
  partition 0:64 / 64:128), exp via ScalarE with mask folded into
  per-partition scale/bias, av accumulation in PSUM.
  normalize: reciprocal_approx_fast on the denominator row, broadcast via
  a K=1 matmul, elementwise mul -> OT; head 1 moved to partitions 64:128
  by an SBUF->SBUF DMA so the O-projection contracts over all 128 dims.
  out_partial[b] = OT[b]^T @ Wo_slice     (bf16 partial, host sums 8)
"""

import math

import numpy as np

B, S, D, H = 4, 2048, 1024, 16
HD = D // H  # 64
NCORES = 8
NEG = -1.0e6
P = 128

_PROG_CACHE = {}


def _patch_tile_drain():
    """The walrus build in this container rejects sem waits attached to the
    Tile end-of-kernel Drain ("Too many sync wait commands" / SIGABRT).
    Replace them with standalone EventSemaphore waits, which it accepts."""
    import concourse.tile as tile
    from concourse.vector_clock import ScopedClock

    if getattr(tile.TileContext, "_drain_patched", False):
        return

    def _drain_and_barrier(self, tick_clock, wait_clock):
        nc = self.nc
        drain_inst = nc.sync.drain()
        wait_clock.add_sem_waits(
            drain_inst.ins, ScopedClock({None: tick_clock.global_clock})
        )
        si = drain_inst.ins.sync_info
        waits = list(si.on_wait) if si is not None and si.on_wait else []
        if waits:
            si.on_wait.clear()
            by_id, by_name = {}, {}
            for h in wait_clock.sems.allocated().values():
                by_id[getattr(h, "id", None)] = h
                by_name[getattr(h, "name", None)] = h
            for w in waits:
                h = by_id.get(w.id) or by_name.get(w.ant_name)
                assert h is not None, f"no handle for sem {w.ant_name} ({w.id})"
                nc.sync.wait_ge(h, w.wait_value)
        nc.all_engine_barrier()
        assert self.sems is not None
        popped = nc._tile_sem_poison_stack.pop()
        assert popped is self._sem_poison
        nc.clear_and_free_semaphores(list(self.sems.allocated().values()))
        nc.all_engine_barrier()

    tile.TileContext._drain_and_barrier = _drain_and_barrier
    tile.TileContext._drain_patched = True


def _split_multi_waits(nc, mybir):
    """This container's walrus rejects instructions carrying more than one
    semaphore wait ("Too many sync wait commands"). Hoist excess waits into
    standalone EventSemaphore instructions on the same engine, inserted
    immediately before the instruction — same-engine stream order preserves
    the semantics exactly."""
    n_ev = 0
    for fn in nc.m.functions:
        for bb in fn.blocks:
            insts = bb.instructions
            out = []
            for inst in insts:
                si = inst.sync_info
                waits = list(si.on_wait) if si is not None and si.on_wait else []
                keep = 0 if inst.opcode == "Drain" else 1
                if len(waits) > keep:
                    excess = waits[: len(waits) - keep]
                    kept = waits[len(waits) - keep:]
                    si.on_wait.clear()
                    si.on_wait.extend(kept)
                    for w in excess:
                        ev = mybir.InstEventSemaphore(
                            name=f"{inst.name}-hw{n_ev}",
                            engine=inst.engine,
                        )
                        ev.sync_info = mybir.SyncInfo(on_wait=[w], on_update=[])
                        out.append(ev)
                        n_ev += 1
                out.append(inst)
            if n_ev:
                insts[:] = out
    return n_ev


def _build_program(kts: tuple):
    import concourse.bass as bass
    import concourse.mybir as mybir
    import concourse.tile as tile

    _patch_tile_drain()

    f32 = mybir.dt.float32
    bf16 = mybir.dt.bfloat16
    AF = mybir.ActivationFunctionType

    KT_total = sum(kts)  # total key tiles across batches
    koff = [0]
    for kt in kts:
        koff.append(koff[-1] + kt)
    SK = KT_total * P  # total truncated key rows

    nc = bass.Bass()

    xq_d = nc.dram_tensor("xqt", [B, D, S], bf16, kind="ExternalInput")
    xk_d = nc.dram_tensor("xkt", [D, SK], bf16, kind="ExternalInput")
    xv_d = nc.dram_tensor("xvt", [D, SK], bf16, kind="ExternalInput")
    wq_d = nc.dram_tensor("wq", [D, P], bf16, kind="ExternalInput")
    wk_d = nc.dram_tensor("wk", [D, P], bf16, kind="ExternalInput")
    wv_d = nc.dram_tensor("wv", [D, P], bf16, kind="ExternalInput")
    wo_d = nc.dram_tensor("wo", [P, D], bf16, kind="ExternalInput")
    mb_d = nc.dram_tensor("mb", [P, KT_total], f32, kind="ExternalInput")
    ms_d = nc.dram_tensor("ms", [P, KT_total], f32, kind="ExternalInput")
    out_d = nc.dram_tensor("out", [B, S, D], bf16, kind="ExternalOutput")

    # process big batches first so their long ScalarE exp streams drain
    # under later batches' PE work
    border = sorted(range(B), key=lambda b: -kts[b])

    with tile.TileContext(nc) as tc:
        with (
            tc.tile_pool(name="pp", bufs=1) as pp,
            tc.tile_pool(name="xp", bufs=2) as xp,
            tc.tile_pool(name="expp", bufs=3) as expp,
            tc.tile_pool(name="dnp", bufs=2) as dnp,
            tc.tile_pool(name="lgp", bufs=2) as lgp,
            tc.tile_pool(name="dnbp", bufs=2) as dnbp,
            tc.tile_pool(name="notp", bufs=2) as notp,
            tc.tile_pool(name="outp", bufs=2) as outp,
            tc.tile_pool(name="psS", bufs=2, space="PSUM") as psS,
            tc.tile_pool(name="psB", bufs=1, space="PSUM") as psB,
            tc.tile_pool(name="psM", bufs=2, space="PSUM") as psM,
        ):
            # persistent tensors
            QT = pp.tile([P, B, S], bf16, name="QT")
            KT = pp.tile([P, SK], bf16, name="KT")
            V = pp.tile([P, KT_total, 2, HD + 1], bf16, name="V")
            OT = pp.tile([P, B, S], bf16, name="OT")
            ones = pp.tile([65, P], bf16, name="ones")
            mb = pp.tile([P, KT_total], f32, name="mb")
            msc = pp.tile([P, KT_total], f32, name="msc")

            nc.any.memset(ones[:], 1.0)
            nc.any.memset(V[:, :, :, HD:HD + 1], 1.0)
            nc.sync.dma_start(mb[:], mb_d[:, :])
            nc.sync.dma_start(msc[:], ms_d[:, :])

            wq = pp.tile([P, 8, P], bf16, name="wq")
            wk = pp.tile([P, 8, P], bf16, name="wk")
            wv = pp.tile([P, 8, P], bf16, name="wv")
            wo = pp.tile([P, D], bf16, name="wo")
            nc.sync.dma_start(wq[:], wq_d[:, :].rearrange("(a p) c -> p a c", p=P))
            nc.sync.dma_start(wk[:], wk_d[:, :].rearrange("(a p) c -> p a c", p=P))
            nc.sync.dma_start(wv[:], wv_d[:, :].rearrange("(a p) c -> p a c", p=P))
            nc.sync.dma_start(wo[:], wo_d[:, :])

            def gen_proj(b):
                """Q/K/V projection units for batch b; yields per unit."""
                ktb = kts[b]
                Kb = ktb * P
                kb0 = koff[b] * P

                xq_re = xq_d[b, :, :].rearrange("(a p) s -> p a s", p=P)
                for sl in range(2):
                    xt = xp.tile([P, 8, 1024], bf16, name="xt", tag="xt")
                    nc.sync.dma_start(
                        xt[:], xq_re[:, :, sl * 1024:(sl + 1) * 1024]
                    )
                    for sub in range(2):
                        q0 = sl * 1024 + sub * 512
                        ps = psM.tile([P, 512], f32, name="ps", tag="M")
                        for a in range(8):
                            nc.tensor.matmul(
                                ps[:],
                                lhsT=wq[:, a, :],
                                rhs=xt[:, a, sub * 512:(sub + 1) * 512],
                                start=(a == 0),
                                stop=(a == 7),
                            )
                        nc.vector.tensor_copy(
                            out=QT[:, b, q0:q0 + 512], in_=ps[:]
                        )
                        yield

                xk_re = xk_d[:, :].rearrange("(a p) s -> p a s", p=P)
                for o in range(0, Kb, 1024):
                    w = min(1024, Kb - o)
                    xt = xp.tile([P, 8, w], bf16, name="xtk", tag="xt")
                    nc.sync.dma_start(xt[:], xk_re[:, :, kb0 + o:kb0 + o + w])
                    for so in range(0, w, 512):
                        sw = min(512, w - so)
                        ps = psM.tile([P, sw], f32, name="psk", tag="M")
                        for a in range(8):
                            nc.tensor.matmul(
                                ps[:],
                                lhsT=wk[:, a, :],
                                rhs=xt[:, a, so:so + sw],
                                start=(a == 0),
                                stop=(a == 7),
                            )
                        nc.vector.tensor_copy(
                            out=KT[:, kb0 + o + so:kb0 + o + so + sw],
                            in_=ps[:],
                        )
                        yield

                xv_re = xv_d[:, :].rearrange("(a p) s -> p a s", p=P)
                for o in range(0, Kb, 1024):
                    w = min(1024, Kb - o)
                    xt = xp.tile([P, 8, w], bf16, name="xtv", tag="xt")
                    nc.sync.dma_start(xt[:], xv_re[:, :, kb0 + o:kb0 + o + w])
                    for loc in range(w // P):
                        gk = koff[b] + (o // P) + loc
                        pv = psM.tile([P, P], f32, name="pv", tag="M")
                        for a in range(8):
                            nc.tensor.matmul(
                                pv[:],
                                lhsT=xt[:, a, loc * P:(loc + 1) * P],
                                rhs=wv[:, a, :],
                                start=(a == 0),
                                stop=(a == 7),
                            )
                        nc.vector.tensor_copy(
                            out=V[:, gk, :, 0:HD],
                            in_=pv[:].rearrange("p (h c) -> p h c", c=HD),
                        )
                        if loc % 2 == 1:
                            yield

            def gen_att(b):
                """Attention for batch b, head-serial; yields per key tile."""
                ktb = kts[b]
                kb0 = koff[b] * P
                for qh in range(2):
                    q0 = qh * 1024
                    for s in range(2):
                        pb = s * HD
                        av = psB.tile([65, 1024], f32, name="av", tag="av")
                        for kt in range(ktb):
                            gk = koff[b] + kt
                            sc = psS.tile([P, 1024], f32, name="sc", tag="S")
                            for qs in range(2):
                                nc.tensor.matmul(
                                    sc[:, qs * 512:(qs + 1) * 512],
                                    lhsT=KT[
                                        pb:pb + HD,
                                        kb0 + kt * P:kb0 + (kt + 1) * P,
                                    ],
                                    rhs=QT[
                                        pb:pb + HD, b,
                                        q0 + qs * 512:q0 + (qs + 1) * 512,
                                    ],
                                    start=True,
                                    stop=True,
                                )
                            ex = expp.tile([P, 1024], bf16, name="ex", tag="ex")
                            nc.scalar.activation(
                                ex[:],
                                sc[:],
                                AF.Exp,
                                bias=mb[:, gk:gk + 1],
                                scale=msc[:, gk:gk + 1],
                            )
                            for qs in range(2):
                                nc.tensor.matmul(
                                    av[:, qs * 512:(qs + 1) * 512],
                                    lhsT=V[:, gk, s, :],
                                    rhs=ex[:, qs * 512:(qs + 1) * 512],
                                    start=(kt == 0),
                                    stop=(kt == ktb - 1),
                                )
                            yield

                        # normalize: OT[64s:64s+64, b, q0:+1024] = av/denom
                        avb = dnp.tile([65, 1024], f32, name="avb", tag="dn")
                        nc.vector.tensor_copy(out=avb[:], in_=av[:])
                        # 1/denom = exp(-ln(denom)) on ScalarE (same table
                        # set as the attention exp; DVE reciprocal is slow)
                        lg = lgp.tile([65, 1024], f32, name="lg", tag="lg")
                        nc.scalar.activation(
                            lg[64:65, :], avb[64:65, :], AF.Ln
                        )
                        dnb = dnbp.tile([65, 1024], bf16, name="dnb", tag="dnb")
                        nc.scalar.activation(
                            dnb[64:65, :], lg[64:65, :], AF.Exp, scale=-1.0
                        )
                        nt = None
                        if s == 1:
                            nt = notp.tile([HD, 1024], bf16, name="nt", tag="nt")
                        for qs in range(2):
                            bc = psM.tile([P, 512], f32, name="bc", tag="M")
                            nc.tensor.matmul(
                                bc[:],
                                lhsT=ones[64:65, :],
                                rhs=dnb[64:65, qs * 512:(qs + 1) * 512],
                                start=True,
                                stop=True,
                            )
                            qq = q0 + qs * 512
                            if s == 0:
                                nc.vector.tensor_mul(
                                    out=OT[0:HD, b, qq:qq + 512],
                                    in0=avb[0:HD, qs * 512:(qs + 1) * 512],
                                    in1=bc[0:HD, :],
                                )
                            else:
                                nc.vector.tensor_mul(
                                    out=nt[:, qs * 512:(qs + 1) * 512],
                                    in0=avb[0:HD, qs * 512:(qs + 1) * 512],
                                    in1=bc[0:HD, :],
                                )
                                if qs == 1:
                                    nc.sync.dma_start(
                                        OT[HD:P, b, q0:q0 + 1024], nt[:]
                                    )
                        yield

            def gen_oproj(b):
                """O-projection chunks for batch b; yields per chunk."""
                for ch in range(16):
                    ob = outp.tile([P, 1024], bf16, name="ob", tag="ob")
                    for e in range(2):
                        po = psM.tile([P, 512], f32, name="po", tag="M")
                        nc.tensor.matmul(
                            po[:],
                            lhsT=OT[:, b, ch * P:(ch + 1) * P],
                            rhs=wo[:, e * 512:(e + 1) * 512],
                            start=True,
                            stop=True,
                        )
                        nc.vector.tensor_copy(
                            out=ob[:, e * 512:(e + 1) * 512], in_=po[:]
                        )
                    nc.sync.dma_start(out_d[b, ch * P:(ch + 1) * P, :], ob[:])
                    yield

            def drain(g):
                for _ in g:
                    pass

            # Software pipeline across batches: while batch i\'s attention
            # runs (ScalarE-paced, PE half idle), emit batch i-1\'s
            # O-projection and batch i+1\'s projections so the PE stream
            # stays dense (HAM stays at full clock) and pool-slot rotation
            # never serializes one phase behind another.
            drain(gen_proj(border[0]))
            for i in range(B):
                a_units = [u for u in [gen_att(border[i])]]
                A = a_units[0]
                Bs = []
                if i > 0:
                    Bs.append(gen_oproj(border[i - 1]))
                if i + 1 < B:
                    Bs.append(gen_proj(border[i + 1]))
                nA = 4 * kts[border[i]] + 8
                nB = (16 if i > 0 else 0) + (
                    (4 + 2 * ((kts[border[i + 1]] * P + 511) // 512)
                     + (kts[border[i + 1]] + 1) // 2) if i + 1 < B else 0
                )
                import itertools
                Bit = itertools.chain(*Bs)
                done_b = 0
                step = 0
                for _ in A:
                    step += 1
                    want = (nB * step) // nA
                    while done_b < want:
                        if next(Bit, None) is None:
                            done_b = nB
                            break
                        done_b += 1
                for _ in Bit:
                    pass
            drain(gen_oproj(border[B - 1]))

    _split_multi_waits(nc, mybir)
    return nc


def _get_program(kts: tuple):
    if kts not in _PROG_CACHE:
        _PROG_CACHE[kts] = _build_program(kts)
    return _PROG_CACHE[kts]


def kernel(**inputs) -> np.ndarray:
    import ml_dtypes
    from concourse.bass_utils import run_bass_kernel_spmd

    bf = ml_dtypes.bfloat16

    q = np.asarray(inputs["queries"], dtype=np.float32)
    k = np.asarray(inputs["keys"], dtype=np.float32)
    v = np.asarray(inputs["values"], dtype=np.float32)
    vl = np.asarray(inputs["valid_lens"]).astype(np.int64)
    Wq = np.asarray(inputs["Wq"], dtype=np.float32)
    Wk = np.asarray(inputs["Wk"], dtype=np.float32)
    Wv = np.asarray(inputs["Wv"], dtype=np.float32)
    Wo = np.asarray(inputs["Wo"], dtype=np.float32)

    kts = tuple(
        S // P if vl[b] == 0 else min(S // P, int(math.ceil(vl[b] / P)))
        for b in range(B)
    )
    KT_total = sum(kts)
    nc = _get_program(kts)

    # shared (batch-level) arrays — identical on every core
    xqt = np.ascontiguousarray(q.transpose(0, 2, 1)).astype(bf)  # [B, D, S]
    xkt = np.concatenate(
        [k[b, : kts[b] * P].T for b in range(B)], axis=1
    ).astype(bf)  # [D, SK]
    xvt = np.concatenate(
        [v[b, : kts[b] * P].T for b in range(B)], axis=1
    ).astype(bf)

    m_bias = np.empty((P, KT_total), np.float32)
    m_scale = np.empty((P, KT_total), np.float32)
    col = 0
    for b in range(B):
        vlb = int(vl[b])
        for j in range(kts[b]):
            kk = j * P + np.arange(P)
            if vlb == 0:
                m_bias[:, col] = 0.0
                m_scale[:, col] = 0.0
            else:
                m_bias[:, col] = np.where(kk < vlb, 0.0, NEG)
                m_scale[:, col] = 1.0 / math.sqrt(HD)
            col += 1

    in_maps = []
    for c in range(NCORES):
        cols = slice(c * P, (c + 1) * P)  # 2 heads = 128 dims
        in_maps.append(
            {
                "xqt": xqt,
                "xkt": xkt,
                "xvt": xvt,
                "wq": np.ascontiguousarray(Wq[:, cols]).astype(bf),
                "wk": np.ascontiguousarray(Wk[:, cols]).astype(bf),
                "wv": np.ascontiguousarray(Wv[:, cols]).astype(bf),
                "wo": np.ascontiguousarray(Wo[cols, :]).astype(bf),
                "mb": m_bias,
                "ms": m_scale,
            }
        )

    globals()["_LAST_IN_MAPS"] = in_maps
    res = run_bass_kernel_spmd(nc, in_maps, list(range(NCORES))).results

    acc = res[0]["out"].astype(np.float32)
    for c in range(1, NCORES):
        acc += res[c]["out"].astype(np.float32)
    return acc


# revision 16
# speedup vs baseline: 1.2404x; 1.2404x over previous
"""Multi-head attention (B=4, S=2048, D=1024, H=16) on 8 TRN2 NeuronCores.

Load-balanced sharding: every core handles heads {2c, 2c+1} of ALL four
batches. Per-batch attention depth kt_b = ceil(valid_len_b / 128) is baked
into the program (identical on every core -> SPMD-safe), so the per-core
work is Sum_b kt_b head-key-tiles regardless of how skewed valid_lens are.
Keys/values are truncated to kt_b*128 rows (masked keys contribute exp=0).

Per-core dataflow (all matmuls bf16 on the PE array):
  QT[b][d',q]   = (Xq[b] Wq_slice)^T      d' = 128 dims of 2 heads
  KT[b][d',k]   = (Xk[b,:Kb] Wk_slice)^T
  V[b][k,s,65]  = Xv[b,:Kb] Wv_slice      (ones column -> softmax denom)
  per (b, q-half, key tile): scores pair (64-dim contraction, heads at
  partition 0:64 / 64:128), exp via ScalarE with mask folded into
  per-partition scale/bias, av accumulation in PSUM.
  normalize: reciprocal_approx_fast on the denominator row, broadcast via
  a K=1 matmul, elementwise mul -> OT; head 1 moved to partitions 64:128
  by an SBUF->SBUF DMA so the O-projection contracts over all 128 dims.
  out_partial[b] = OT[b]^T @ Wo_slice     (bf16 partial, host sums 8)
"""

import math

import numpy as np

B, S, D, H = 4, 2048, 1024, 16
HD = D // H  # 64
NCORES = 8
NEG = -1.0e6
P = 128

_PROG_CACHE = {}


def _patch_tile_drain():
    """The walrus build in this container rejects sem waits attached to the
    Tile end-of-kernel Drain ("Too many sync wait commands" / SIGABRT).
    Replace them with standalone EventSemaphore waits, which it accepts."""
    import concourse.tile as tile
    from concourse.vector_clock import ScopedClock

    if getattr(tile.TileContext, "_drain_patched", False):
        return

    def _drain_and_barrier(self, tick_clock, wait_clock):
        nc = self.nc
        drain_inst = nc.sync.drain()
        wait_clock.add_sem_waits(
            drain_inst.ins, ScopedClock({None: tick_clock.global_clock})
        )
        si = drain_inst.ins.sync_info
        waits = list(si.on_wait) if si is not None and si.on_wait else []
        if waits:
            si.on_wait.clear()
            by_id, by_name = {}, {}
            for h in wait_clock.sems.allocated().values():
                by_id[getattr(h, "id", None)] = h
                by_name[getattr(h, "name", None)] = h
            for w in waits:
                h = by_id.get(w.id) or by_name.get(w.ant_name)
                assert h is not None, f"no handle for sem {w.ant_name} ({w.id})"
                nc.sync.wait_ge(h, w.wait_value)
        nc.all_engine_barrier()
        assert self.sems is not None
        popped = nc._tile_sem_poison_stack.pop()
        assert popped is self._sem_poison
        nc.clear_and_free_semaphores(list(self.sems.allocated().values()))
        nc.all_engine_barrier()

    tile.TileContext._drain_and_barrier = _drain_and_barrier
    tile.TileContext._drain_patched = True


def _split_multi_waits(nc, mybir):
    """This container's walrus rejects instructions carrying more than one
    semaphore wait ("Too many sync wait commands"). Hoist excess waits into
    standalone EventSemaphore instructions on the same engine, inserted
    immediately before the instruction — same-engine stream order preserves
    the semantics exactly."""
    n_ev = 0
    for fn in nc.m.functions:
        for bb in fn.blocks:
            insts = bb.instructions
            out = []
            for inst in insts:
                si = inst.sync_info
                waits = list(si.on_wait) if si is not None and si.on_wait else []
                keep = 0 if inst.opcode == "Drain" else 1
                if len(waits) > keep:
                    excess = waits[: len(waits) - keep]
                    kept = waits[len(waits) - keep:]
                    si.on_wait.clear()
                    si.on_wait.extend(kept)
                    for w in excess:
                        ev = mybir.InstEventSemaphore(
                            name=f"{inst.name}-hw{n_ev}",
                            engine=inst.engine,
                        )
                        ev.sync_info = mybir.SyncInfo(on_wait=[w], on_update=[])
                        out.append(ev)
                        n_ev += 1
                out.append(inst)
            if n_ev:
                insts[:] = out
    return n_ev


def _build_program(kts: tuple):
    import concourse.bass as bass
    import concourse.mybir as mybir
    import concourse.tile as tile

    _patch_tile_drain()

    f32 = mybir.dt.float32
    bf16 = mybir.dt.bfloat16
    f8 = mybir.dt.float8e4
    AF = mybir.ActivationFunctionType
    DR = mybir.MatmulPerfMode.DoubleRow

    KT_total = sum(kts)  # total key tiles across batches
    koff = [0]
    for kt in kts:
        koff.append(koff[-1] + kt)
    SK = KT_total * P  # total truncated key rows
    # fp8 DoubleRow processes key tiles in pairs; odd batches get a bf16 tail
    npair = [k // 2 for k in kts]
    poff = [0]
    for n in npair:
        poff.append(poff[-1] + n)
    NP = max(1, poff[-1])

    nc = bass.Bass()

    xq_d = nc.dram_tensor("xqt", [B, D, S], bf16, kind="ExternalInput")
    xk_d = nc.dram_tensor("xkt", [D, SK], bf16, kind="ExternalInput")
    xv_d = nc.dram_tensor("xvt", [D, SK], bf16, kind="ExternalInput")
    wq_d = nc.dram_tensor("wq", [D, P], bf16, kind="ExternalInput")
    wk_d = nc.dram_tensor("wk", [D, P], bf16, kind="ExternalInput")
    wv_d = nc.dram_tensor("wv", [D, P], bf16, kind="ExternalInput")
    wo_d = nc.dram_tensor("wo", [P, D], bf16, kind="ExternalInput")
    mb_d = nc.dram_tensor("mb", [P, KT_total], f32, kind="ExternalInput")
    ms_d = nc.dram_tensor("ms", [P, KT_total], f32, kind="ExternalInput")
    out_d = nc.dram_tensor("out", [B, S, D], bf16, kind="ExternalOutput")

    # process big batches first so their long ScalarE exp streams drain
    # under later batches' PE work
    border = sorted(range(B), key=lambda b: -kts[b])

    with tile.TileContext(nc) as tc:
        with (
            tc.tile_pool(name="pp", bufs=1) as pp,
            tc.tile_pool(name="xp", bufs=3) as xp,
            tc.tile_pool(name="expp", bufs=3) as expp,
            tc.tile_pool(name="dnp", bufs=2) as dnp,
            tc.tile_pool(name="lgp", bufs=2) as lgp,
            tc.tile_pool(name="dnbp", bufs=2) as dnbp,
            tc.tile_pool(name="notp", bufs=2) as notp,
            tc.tile_pool(name="outp", bufs=2) as outp,
            tc.tile_pool(name="psS", bufs=2, space="PSUM") as psS,
            tc.tile_pool(name="psB", bufs=1, space="PSUM") as psB,
            tc.tile_pool(name="psM", bufs=2, space="PSUM") as psM,
        ):
            # persistent tensors
            QT = pp.tile([P, B, S], bf16, name="QT")
            KT = pp.tile([P, SK], bf16, name="KT")
            V = pp.tile([P, KT_total, 2, HD + 1], bf16, name="V")
            OT = pp.tile([P, B, S], bf16, name="OT")
            ones = pp.tile([65, P], bf16, name="ones")
            mb = pp.tile([P, KT_total], f32, name="mb")
            msc = pp.tile([P, KT_total], f32, name="msc")

            nc.any.memset(ones[:], 1.0)
            nc.any.memset(V[:, :, :, HD:HD + 1], 1.0)
            nc.sync.dma_start(mb[:], mb_d[:, :])
            nc.sync.dma_start(msc[:], ms_d[:, :])

            wq = pp.tile([P, 8, P], bf16, name="wq")
            wk = pp.tile([P, 8, P], bf16, name="wk")
            wv = pp.tile([P, 8, P], bf16, name="wv")
            wo = pp.tile([P, D], bf16, name="wo")
            nc.sync.dma_start(wq[:], wq_d[:, :].rearrange("(a p) c -> p a c", p=P))
            nc.sync.dma_start(wk[:], wk_d[:, :].rearrange("(a p) c -> p a c", p=P))
            nc.sync.dma_start(wv[:], wv_d[:, :].rearrange("(a p) c -> p a c", p=P))
            nc.sync.dma_start(wo[:], wo_d[:, :])

            def gen_proj(b):
                """Q/K/V projection units for batch b; yields per unit."""
                ktb = kts[b]
                Kb = ktb * P
                kb0 = koff[b] * P

                xq_re = xq_d[b, :, :].rearrange("(a p) s -> p a s", p=P)
                for sl in range(2):
                    xt = xp.tile([P, 8, 1024], bf16, name="xt", tag="xt")
                    nc.sync.dma_start(
                        xt[:], xq_re[:, :, sl * 1024:(sl + 1) * 1024]
                    )
                    for sub in range(2):
                        q0 = sl * 1024 + sub * 512
                        ps = psM.tile([P, 512], f32, name="ps", tag="M")
                        for a in range(8):
                            nc.tensor.matmul(
                                ps[:],
                                lhsT=wq[:, a, :],
                                rhs=xt[:, a, sub * 512:(sub + 1) * 512],
                                start=(a == 0),
                                stop=(a == 7),
                            )
                        nc.vector.tensor_copy(
                            out=QT[:, b, q0:q0 + 512], in_=ps[:]
                        )
                        yield

                xk_re = xk_d[:, :].rearrange("(a p) s -> p a s", p=P)
                for o in range(0, Kb, 1024):
                    w = min(1024, Kb - o)
                    xt = xp.tile([P, 8, w], bf16, name="xtk", tag="xt")
                    nc.sync.dma_start(xt[:], xk_re[:, :, kb0 + o:kb0 + o + w])
                    for so in range(0, w, 512):
                        sw = min(512, w - so)
                        ps = psM.tile([P, sw], f32, name="psk", tag="M")
                        for a in range(8):
                            nc.tensor.matmul(
                                ps[:],
                                lhsT=wk[:, a, :],
                                rhs=xt[:, a, so:so + sw],
                                start=(a == 0),
                                stop=(a == 7),
                            )
                        nc.vector.tensor_copy(
                            out=KT[:, kb0 + o + so:kb0 + o + so + sw],
                            in_=ps[:],
                        )
                        yield

                xv_re = xv_d[:, :].rearrange("(a p) s -> p a s", p=P)
                for o in range(0, Kb, 1024):
                    w = min(1024, Kb - o)
                    xt = xp.tile([P, 8, w], bf16, name="xtv", tag="xt")
                    nc.sync.dma_start(xt[:], xv_re[:, :, kb0 + o:kb0 + o + w])
                    for loc in range(w // P):
                        kt = (o // P) + loc
                        pv = psM.tile([P, P], f32, name="pv", tag="M")
                        for a in range(8):
                            nc.tensor.matmul(
                                pv[:],
                                lhsT=xt[:, a, loc * P:(loc + 1) * P],
                                rhs=wv[:, a, :],
                                start=(a == 0),
                                stop=(a == 7),
                            )
                        nc.vector.tensor_copy(
                            out=V[:, koff[b] + kt, :, 0:HD],
                            in_=pv[:].rearrange("p (h c) -> p h c", c=HD),
                        )
                        if loc % 2 == 1:
                            yield

            def gen_att_qh(b, qh):
                """Attention for batch b, one q-half; yields per key tile."""
                ktb = kts[b]
                kb0 = koff[b] * P
                q0 = qh * 1024
                for s in range(2):
                    pb = s * HD
                    av = psB.tile([65, 1024], f32, name="av", tag="av")

                    def scores(kt):
                        gk = koff[b] + kt
                        sc = psS.tile([P, 1024], f32, name="sc", tag="S")
                        for qs in range(2):
                            nc.tensor.matmul(
                                sc[:, qs * 512:(qs + 1) * 512],
                                lhsT=KT[
                                    pb:pb + HD,
                                    kb0 + kt * P:kb0 + (kt + 1) * P,
                                ],
                                rhs=QT[
                                    pb:pb + HD, b,
                                    q0 + qs * 512:q0 + (qs + 1) * 512,
                                ],
                                start=True,
                                stop=True,
                            )
                        return sc, gk

                    for kt in range(ktb):
                        sc, gk = scores(kt)
                        ex = expp.tile([P, 1024], bf16, name="ex", tag="ex")
                        nc.scalar.activation(
                            ex[:],
                            sc[:],
                            AF.Exp,
                            bias=mb[:, gk:gk + 1],
                            scale=msc[:, gk:gk + 1],
                        )
                        for qs in range(2):
                            nc.tensor.matmul(
                                av[:, qs * 512:(qs + 1) * 512],
                                lhsT=V[:, koff[b] + kt, s, :],
                                rhs=ex[:, qs * 512:(qs + 1) * 512],
                                start=(kt == 0),
                                stop=(kt == ktb - 1),
                            )
                        yield

                    # normalize: OT[64s:64s+64, b, q0:+1024] = av/denom
                    avb = dnp.tile([65, 1024], f32, name="avb", tag="dn")
                    nc.vector.tensor_copy(out=avb[:], in_=av[:])
                    # 1/denom = exp(-ln(denom)) on ScalarE (same table
                    # set as the attention exp; DVE reciprocal is slow)
                    lg = lgp.tile([65, 1024], f32, name="lg", tag="lg")
                    nc.scalar.activation(
                        lg[64:65, :], avb[64:65, :], AF.Ln
                    )
                    dnb = dnbp.tile([65, 1024], bf16, name="dnb", tag="dnb")
                    nc.scalar.activation(
                        dnb[64:65, :], lg[64:65, :], AF.Exp, scale=-1.0
                    )
                    nt = None
                    if s == 1:
                        nt = notp.tile([HD, 1024], bf16, name="nt", tag="nt")
                    for qs in range(2):
                        bc = psM.tile([P, 512], f32, name="bc", tag="M")
                        nc.tensor.matmul(
                            bc[:],
                            lhsT=ones[64:65, :],
                            rhs=dnb[64:65, qs * 512:(qs + 1) * 512],
                            start=True,
                            stop=True,
                        )
                        qq = q0 + qs * 512
                        if s == 0:
                            nc.vector.tensor_mul(
                                out=OT[0:HD, b, qq:qq + 512],
                                in0=avb[0:HD, qs * 512:(qs + 1) * 512],
                                in1=bc[0:HD, :],
                            )
                        else:
                            nc.vector.tensor_mul(
                                out=nt[:, qs * 512:(qs + 1) * 512],
                                in0=avb[0:HD, qs * 512:(qs + 1) * 512],
                                in1=bc[0:HD, :],
                            )
                            if qs == 1:
                                nc.sync.dma_start(
                                    OT[HD:P, b, q0:q0 + 1024], nt[:]
                                )
                    yield

            def gen_oproj(b, half, on_act):
                """O-projection chunks for one q-half of batch b."""
                for ch in range(8 * half, 8 * half + 8):
                    ob = outp.tile([P, 1024], bf16, name="ob", tag="ob")
                    for e in range(2):
                        po = psM.tile([P, 512], f32, name="po", tag="M")
                        nc.tensor.matmul(
                            po[:],
                            lhsT=OT[:, b, ch * P:(ch + 1) * P],
                            rhs=wo[:, e * 512:(e + 1) * 512],
                            start=True,
                            stop=True,
                        )
                        if on_act and (ch + e) % 2 == 0:
                            nc.scalar.copy(
                                out=ob[:, e * 512:(e + 1) * 512], in_=po[:]
                            )
                        else:
                            nc.vector.tensor_copy(
                                out=ob[:, e * 512:(e + 1) * 512], in_=po[:]
                            )
                    nc.sync.dma_start(out_d[b, ch * P:(ch + 1) * P, :], ob[:])
                    yield

            # Software pipeline across batches: while batch i attention
            # runs (ScalarE-paced), emit the previous batch O-projection
            # and the next batch projections so the PE stream stays dense
            # (HAM stays at full clock) and pool-slot rotation never
            # serializes one phase behind another.
            def n_att_units(b):
                return kts[b] + 1  # per (qh, s): +1 norm

            def drive(A, nA, Bunits, nB):
                Bit = iter(Bunits)
                done_b = 0
                step = 0
                for _ in A:
                    step += 1
                    want = (nB * step) // max(1, nA)
                    while done_b < want:
                        if next(Bit, None) is None:
                            done_b = nB
                            break
                        done_b += 1
                for _ in Bit:
                    pass

            import itertools

            def proj_count(b):
                ktb = kts[b]
                Kb = ktb * P
                n = 4  # Q spans
                for o in range(0, Kb, 1024):
                    w = min(1024, Kb - o)
                    n += (w + 511) // 512  # K spans
                    n += (w // P + 1) // 2  # V (yields every 2 ktiles)
                return n

            for _ in gen_proj(border[0]):
                pass
            for i in range(B):
                bcur = border[i]
                for qh in range(2):
                    A = gen_att_qh(bcur, qh)
                    nA = 2 * n_att_units(bcur)
                    Bs = []
                    nB = 0
                    if qh == 0:
                        if i > 0:
                            Bs.append(gen_oproj(border[i - 1], 1, i >= 2))
                            nB += 8
                        if i + 1 < B:
                            g = gen_proj(border[i + 1])
                            Bs.append(g)
                            nB += proj_count(border[i + 1])
                    else:
                        Bs.append(gen_oproj(bcur, 0, i >= 2))
                        nB += 8
                    drive(A, nA, itertools.chain(*Bs), nB)
            for _ in gen_oproj(border[B - 1], 1, True):
                pass

    _split_multi_waits(nc, mybir)
    return nc


def _get_program(kts: tuple):
    if kts not in _PROG_CACHE:
        _PROG_CACHE[kts] = _build_program(kts)
    return _PROG_CACHE[kts]


def kernel(**inputs) -> np.ndarray:
    import ml_dtypes
    from concourse.bass_utils import run_bass_kernel_spmd

    bf = ml_dtypes.bfloat16

    q = np.asarray(inputs["queries"], dtype=np.float32)
    k = np.asarray(inputs["keys"], dtype=np.float32)
    v = np.asarray(inputs["values"], dtype=np.float32)
    vl = np.asarray(inputs["valid_lens"]).astype(np.int64)
    Wq = np.asarray(inputs["Wq"], dtype=np.float32)
    Wk = np.asarray(inputs["Wk"], dtype=np.float32)
    Wv = np.asarray(inputs["Wv"], dtype=np.float32)
    Wo = np.asarray(inputs["Wo"], dtype=np.float32)

    kts = tuple(
        S // P if vl[b] == 0 else min(S // P, int(math.ceil(vl[b] / P)))
        for b in range(B)
    )
    KT_total = sum(kts)
    nc = _get_program(kts)

    # shared (batch-level) arrays — identical on every core
    xqt = np.ascontiguousarray(q.transpose(0, 2, 1)).astype(bf)  # [B, D, S]
    xkt = np.concatenate(
        [k[b, : kts[b] * P].T for b in range(B)], axis=1
    ).astype(bf)  # [D, SK]
    xvt = np.concatenate(
        [v[b, : kts[b] * P].T for b in range(B)], axis=1
    ).astype(bf)

    m_bias = np.empty((P, KT_total), np.float32)
    m_scale = np.empty((P, KT_total), np.float32)
    col = 0
    for b in range(B):
        vlb = int(vl[b])
        for j in range(kts[b]):
            kk = j * P + np.arange(P)
            if vlb == 0:
                m_bias[:, col] = 0.0
                m_scale[:, col] = 0.0
            else:
                m_bias[:, col] = np.where(kk < vlb, 0.0, NEG)
                m_scale[:, col] = 1.0 / math.sqrt(HD)
            col += 1

    in_maps = []
    for c in range(NCORES):
        cols = slice(c * P, (c + 1) * P)  # 2 heads = 128 dims
        in_maps.append(
            {
                "xqt": xqt,
                "xkt": xkt,
                "xvt": xvt,
                "wq": np.ascontiguousarray(Wq[:, cols]).astype(bf),
                "wk": np.ascontiguousarray(Wk[:, cols]).astype(bf),
                "wv": np.ascontiguousarray(Wv[:, cols]).astype(bf),
                "wo": np.ascontiguousarray(Wo[cols, :]).astype(bf),
                "mb": m_bias,
                "ms": m_scale,
            }
        )

    globals()["_LAST_IN_MAPS"] = in_maps
    res = run_bass_kernel_spmd(nc, in_maps, list(range(NCORES))).results

    acc = res[0]["out"].astype(np.float32)
    for c in range(1, NCORES):
        acc += res[c]["out"].astype(np.float32)
    return acc


# revision 17
# speedup vs baseline: 1.2513x; 1.0088x over previous
"""Multi-head attention (B=4, S=2048, D=1024, H=16) on 8 TRN2 NeuronCores.

Load-balanced sharding: every core handles heads {2c, 2c+1} of ALL four
batches. Per-batch attention depth kt_b = ceil(valid_len_b / 128) is baked
into the program (identical on every core -> SPMD-safe), so the per-core
work is Sum_b kt_b head-key-tiles regardless of how skewed valid_lens are.
Keys/values are truncated to kt_b*128 rows (masked keys contribute exp=0).

Per-core dataflow (all matmuls bf16 on the PE array):
  QT[b][d',q]   = (Xq[b] Wq_slice)^T      d' = 128 dims of 2 heads
  KT[b][d',k]   = (Xk[b,:Kb] Wk_slice)^T
  V[b][k,s,65]  = Xv[b,:Kb] Wv_slice      (ones column -> softmax denom)
  per (b, q-half, key tile): scores pair (64-dim contraction, heads at
  partition 0:64 / 64:128), exp via ScalarE with mask folded into
  per-partition scale/bias, av accumulation in PSUM.
  normalize: reciprocal_approx_fast on the denominator row, broadcast via
  a K=1 matmul, elementwise mul -> OT; head 1 moved to partitions 64:128
  by an SBUF->SBUF DMA so the O-projection contracts over all 128 dims.
  out_partial[b] = OT[b]^T @ Wo_slice     (bf16 partial, host sums 8)
"""

import math

import numpy as np

B, S, D, H = 4, 2048, 1024, 16
HD = D // H  # 64
NCORES = 8
NEG = -1.0e6
P = 128

_PROG_CACHE = {}


def _patch_tile_drain():
    """The walrus build in this container rejects sem waits attached to the
    Tile end-of-kernel Drain ("Too many sync wait commands" / SIGABRT).
    Replace them with standalone EventSemaphore waits, which it accepts."""
    import concourse.tile as tile
    from concourse.vector_clock import ScopedClock

    if getattr(tile.TileContext, "_drain_patched", False):
        return

    def _drain_and_barrier(self, tick_clock, wait_clock):
        nc = self.nc
        drain_inst = nc.sync.drain()
        wait_clock.add_sem_waits(
            drain_inst.ins, ScopedClock({None: tick_clock.global_clock})
        )
        si = drain_inst.ins.sync_info
        waits = list(si.on_wait) if si is not None and si.on_wait else []
        if waits:
            si.on_wait.clear()
            by_id, by_name = {}, {}
            for h in wait_clock.sems.allocated().values():
                by_id[getattr(h, "id", None)] = h
                by_name[getattr(h, "name", None)] = h
            for w in waits:
                h = by_id.get(w.id) or by_name.get(w.ant_name)
                assert h is not None, f"no handle for sem {w.ant_name} ({w.id})"
                nc.sync.wait_ge(h, w.wait_value)
        nc.all_engine_barrier()
        assert self.sems is not None
        popped = nc._tile_sem_poison_stack.pop()
        assert popped is self._sem_poison
        nc.clear_and_free_semaphores(list(self.sems.allocated().values()))
        nc.all_engine_barrier()

    tile.TileContext._drain_and_barrier = _drain_and_barrier
    tile.TileContext._drain_patched = True


def _split_multi_waits(nc, mybir):
    """This container's walrus rejects instructions carrying more than one
    semaphore wait ("Too many sync wait commands"). Hoist excess waits into
    standalone EventSemaphore instructions on the same engine, inserted
    immediately before the instruction — same-engine stream order preserves
    the semantics exactly."""
    n_ev = 0
    for fn in nc.m.functions:
        for bb in fn.blocks:
            insts = bb.instructions
            out = []
            for inst in insts:
                si = inst.sync_info
                waits = list(si.on_wait) if si is not None and si.on_wait else []
                keep = 0 if inst.opcode == "Drain" else 1
                if len(waits) > keep:
                    excess = waits[: len(waits) - keep]
                    kept = waits[len(waits) - keep:]
                    si.on_wait.clear()
                    si.on_wait.extend(kept)
                    for w in excess:
                        ev = mybir.InstEventSemaphore(
                            name=f"{inst.name}-hw{n_ev}",
                            engine=inst.engine,
                        )
                        ev.sync_info = mybir.SyncInfo(on_wait=[w], on_update=[])
                        out.append(ev)
                        n_ev += 1
                out.append(inst)
            if n_ev:
                insts[:] = out
    return n_ev


def _build_program(kts: tuple):
    import concourse.bass as bass
    import concourse.mybir as mybir
    import concourse.tile as tile

    _patch_tile_drain()

    f32 = mybir.dt.float32
    bf16 = mybir.dt.bfloat16
    f8 = mybir.dt.float8e4
    AF = mybir.ActivationFunctionType
    DR = mybir.MatmulPerfMode.DoubleRow

    KT_total = sum(kts)  # total key tiles across batches
    koff = [0]
    for kt in kts:
        koff.append(koff[-1] + kt)
    SK = KT_total * P  # total truncated key rows
    # fp8 DoubleRow processes key tiles in pairs; odd batches get a bf16 tail
    npair = [k // 2 for k in kts]
    poff = [0]
    for n in npair:
        poff.append(poff[-1] + n)
    NP = max(1, poff[-1])

    nc = bass.Bass()

    xq_d = nc.dram_tensor("xqt", [B, D, S], bf16, kind="ExternalInput")
    xk_d = nc.dram_tensor("xkt", [D, SK], bf16, kind="ExternalInput")
    xv_d = nc.dram_tensor("xvt", [D, SK], bf16, kind="ExternalInput")
    wq_d = nc.dram_tensor("wq", [D, P], bf16, kind="ExternalInput")
    wk_d = nc.dram_tensor("wk", [D, P], bf16, kind="ExternalInput")
    wv_d = nc.dram_tensor("wv", [D, P], bf16, kind="ExternalInput")
    wo_d = nc.dram_tensor("wo", [P, D], bf16, kind="ExternalInput")
    mb_d = nc.dram_tensor("mb", [P, KT_total], f32, kind="ExternalInput")
    ms_d = nc.dram_tensor("ms", [P, KT_total], f32, kind="ExternalInput")
    out_d = nc.dram_tensor("out", [B, S, D], bf16, kind="ExternalOutput")

    # process big batches first so their long ScalarE exp streams drain
    # under later batches' PE work
    border = sorted(range(B), key=lambda b: -kts[b])

    with tile.TileContext(nc) as tc:
        with (
            tc.tile_pool(name="pp", bufs=1) as pp,
            tc.tile_pool(name="xp", bufs=3) as xp,
            tc.tile_pool(name="expp", bufs=3) as expp,
            tc.tile_pool(name="dnp", bufs=2) as dnp,
            tc.tile_pool(name="dntp", bufs=2) as dntp,
            tc.tile_pool(name="dnbp", bufs=2) as dnbp,
            tc.tile_pool(name="drp", bufs=2, space="DRAM") as drp,
            tc.tile_pool(name="notp", bufs=2) as notp,
            tc.tile_pool(name="outp", bufs=2) as outp,
            tc.tile_pool(name="psS", bufs=2, space="PSUM") as psS,
            tc.tile_pool(name="psB", bufs=1, space="PSUM") as psB,
            tc.tile_pool(name="psM", bufs=2, space="PSUM") as psM,
        ):
            # persistent tensors
            QT = pp.tile([P, B, S], bf16, name="QT")
            KT = pp.tile([P, SK], bf16, name="KT")
            V = pp.tile([P, KT_total, 2, HD + 1], bf16, name="V")
            OT = pp.tile([P, B, S], bf16, name="OT")
            ones = pp.tile([65, P], bf16, name="ones")
            mb = pp.tile([P, KT_total], f32, name="mb")
            msc = pp.tile([P, KT_total], f32, name="msc")

            nc.any.memset(ones[:], 1.0)
            nc.any.memset(V[:, :, :, HD:HD + 1], 1.0)
            nc.sync.dma_start(mb[:], mb_d[:, :])
            nc.sync.dma_start(msc[:], ms_d[:, :])

            wq = pp.tile([P, 8, P], bf16, name="wq")
            wk = pp.tile([P, 8, P], bf16, name="wk")
            wv = pp.tile([P, 8, P], bf16, name="wv")
            wo = pp.tile([P, D], bf16, name="wo")
            nc.sync.dma_start(wq[:], wq_d[:, :].rearrange("(a p) c -> p a c", p=P))
            nc.sync.dma_start(wk[:], wk_d[:, :].rearrange("(a p) c -> p a c", p=P))
            nc.sync.dma_start(wv[:], wv_d[:, :].rearrange("(a p) c -> p a c", p=P))
            nc.sync.dma_start(wo[:], wo_d[:, :])

            def gen_proj(b):
                """Q/K/V projection units for batch b; yields per unit."""
                ktb = kts[b]
                Kb = ktb * P
                kb0 = koff[b] * P

                xq_re = xq_d[b, :, :].rearrange("(a p) s -> p a s", p=P)
                for sl in range(2):
                    xt = xp.tile([P, 8, 1024], bf16, name="xt", tag="xt")
                    nc.sync.dma_start(
                        xt[:], xq_re[:, :, sl * 1024:(sl + 1) * 1024]
                    )
                    for sub in range(2):
                        q0 = sl * 1024 + sub * 512
                        ps = psM.tile([P, 512], f32, name="ps", tag="M")
                        for a in range(8):
                            nc.tensor.matmul(
                                ps[:],
                                lhsT=wq[:, a, :],
                                rhs=xt[:, a, sub * 512:(sub + 1) * 512],
                                start=(a == 0),
                                stop=(a == 7),
                            )
                        nc.vector.tensor_copy(
                            out=QT[:, b, q0:q0 + 512], in_=ps[:]
                        )
                        yield

                xk_re = xk_d[:, :].rearrange("(a p) s -> p a s", p=P)
                for o in range(0, Kb, 1024):
                    w = min(1024, Kb - o)
                    xt = xp.tile([P, 8, w], bf16, name="xtk", tag="xt")
                    nc.sync.dma_start(xt[:], xk_re[:, :, kb0 + o:kb0 + o + w])
                    for so in range(0, w, 512):
                        sw = min(512, w - so)
                        ps = psM.tile([P, sw], f32, name="psk", tag="M")
                        for a in range(8):
                            nc.tensor.matmul(
                                ps[:],
                                lhsT=wk[:, a, :],
                                rhs=xt[:, a, so:so + sw],
                                start=(a == 0),
                                stop=(a == 7),
                            )
                        nc.vector.tensor_copy(
                            out=KT[:, kb0 + o + so:kb0 + o + so + sw],
                            in_=ps[:],
                        )
                        yield

                xv_re = xv_d[:, :].rearrange("(a p) s -> p a s", p=P)
                for o in range(0, Kb, 1024):
                    w = min(1024, Kb - o)
                    xt = xp.tile([P, 8, w], bf16, name="xtv", tag="xt")
                    nc.sync.dma_start(xt[:], xv_re[:, :, kb0 + o:kb0 + o + w])
                    for loc in range(w // P):
                        kt = (o // P) + loc
                        pv = psM.tile([P, P], f32, name="pv", tag="M")
                        for a in range(8):
                            nc.tensor.matmul(
                                pv[:],
                                lhsT=xt[:, a, loc * P:(loc + 1) * P],
                                rhs=wv[:, a, :],
                                start=(a == 0),
                                stop=(a == 7),
                            )
                        nc.vector.tensor_copy(
                            out=V[:, koff[b] + kt, :, 0:HD],
                            in_=pv[:].rearrange("p (h c) -> p h c", c=HD),
                        )
                        if loc % 2 == 1:
                            yield

            def gen_att_qh(b, qh):
                """Attention for batch b, one q-half; yields per key tile."""
                ktb = kts[b]
                kb0 = koff[b] * P
                q0 = qh * 1024
                for s in range(2):
                    pb = s * HD
                    av = psB.tile([65, 1024], f32, name="av", tag="av")

                    def scores(kt):
                        gk = koff[b] + kt
                        sc = psS.tile([P, 1024], f32, name="sc", tag="S")
                        for qs in range(2):
                            nc.tensor.matmul(
                                sc[:, qs * 512:(qs + 1) * 512],
                                lhsT=KT[
                                    pb:pb + HD,
                                    kb0 + kt * P:kb0 + (kt + 1) * P,
                                ],
                                rhs=QT[
                                    pb:pb + HD, b,
                                    q0 + qs * 512:q0 + (qs + 1) * 512,
                                ],
                                start=True,
                                stop=True,
                            )
                        return sc, gk

                    for kt in range(ktb):
                        sc, gk = scores(kt)
                        ex = expp.tile([P, 1024], bf16, name="ex", tag="ex")
                        nc.scalar.activation(
                            ex[:],
                            sc[:],
                            AF.Exp,
                            bias=mb[:, gk:gk + 1],
                            scale=msc[:, gk:gk + 1],
                        )
                        for qs in range(2):
                            nc.tensor.matmul(
                                av[:, qs * 512:(qs + 1) * 512],
                                lhsT=V[:, koff[b] + kt, s, :],
                                rhs=ex[:, qs * 512:(qs + 1) * 512],
                                start=(kt == 0),
                                stop=(kt == ktb - 1),
                            )
                        yield

                    # normalize: OT[64s:64s+64, b, q0:+1024] = av/denom
                    avb = dnp.tile([65, 1024], f32, name="avb", tag="dn")
                    nc.vector.tensor_copy(out=avb[:], in_=av[:])
                    # reciprocal of the denominator row: spread the 1024
                    # values across 128 partitions via a DRAM bounce so the
                    # DVE reciprocal costs 8 elems/lane instead of 1024
                    d1 = drp.tile([1, 1024], f32, name="d1", tag="d1")
                    nc.sync.dma_start(d1[:, :], avb[64:65, :])
                    dnt = dntp.tile([P, 8], f32, name="dnt", tag="dnt")
                    nc.sync.dma_start(
                        dnt[:], d1[0, :].rearrange("(p c) -> p c", p=P)
                    )
                    dnr = dntp.tile([P, 8], bf16, name="dnr", tag="dnr")
                    with nc.allow_low_precision(
                        reason="softmax denominators are O(1e2-1e3); bf16 "
                        "reciprocal keeps enough digits for attention"
                    ):
                        nc.vector.reciprocal(dnr[:], dnt[:])
                    d2 = drp.tile([1, 1024], bf16, name="d2", tag="d2")
                    nc.sync.dma_start(
                        d2[0, :].rearrange("(p c) -> p c", p=P), dnr[:]
                    )
                    dnb = dnbp.tile([65, 1024], bf16, name="dnb", tag="dnb")
                    nc.sync.dma_start(dnb[64:65, :], d2[:, :])
                    nt = None
                    if s == 1:
                        nt = notp.tile([HD, 1024], bf16, name="nt", tag="nt")
                    for qs in range(2):
                        bc = psM.tile([P, 512], f32, name="bc", tag="M")
                        nc.tensor.matmul(
                            bc[:],
                            lhsT=ones[64:65, :],
                            rhs=dnb[64:65, qs * 512:(qs + 1) * 512],
                            start=True,
                            stop=True,
                        )
                        qq = q0 + qs * 512
                        if s == 0:
                            nc.vector.tensor_mul(
                                out=OT[0:HD, b, qq:qq + 512],
                                in0=avb[0:HD, qs * 512:(qs + 1) * 512],
                                in1=bc[0:HD, :],
                            )
                        else:
                            nc.vector.tensor_mul(
                                out=nt[:, qs * 512:(qs + 1) * 512],
                                in0=avb[0:HD, qs * 512:(qs + 1) * 512],
                                in1=bc[0:HD, :],
                            )
                            if qs == 1:
                                nc.sync.dma_start(
                                    OT[HD:P, b, q0:q0 + 1024], nt[:]
                                )
                    yield

            def gen_oproj(b, half, on_act):
                """O-projection chunks for one q-half of batch b."""
                for ch in range(8 * half, 8 * half + 8):
                    ob = outp.tile([P, 1024], bf16, name="ob", tag="ob")
                    for e in range(2):
                        po = psM.tile([P, 512], f32, name="po", tag="M")
                        nc.tensor.matmul(
                            po[:],
                            lhsT=OT[:, b, ch * P:(ch + 1) * P],
                            rhs=wo[:, e * 512:(e + 1) * 512],
                            start=True,
                            stop=True,
                        )
                        if on_act and (ch + e) % 2 == 0:
                            nc.scalar.copy(
                                out=ob[:, e * 512:(e + 1) * 512], in_=po[:]
                            )
                        else:
                            nc.vector.tensor_copy(
                                out=ob[:, e * 512:(e + 1) * 512], in_=po[:]
                            )
                    nc.sync.dma_start(out_d[b, ch * P:(ch + 1) * P, :], ob[:])
                    yield

            # Software pipeline across batches: while batch i attention
            # runs (ScalarE-paced), emit the previous batch O-projection
            # and the next batch projections so the PE stream stays dense
            # (HAM stays at full clock) and pool-slot rotation never
            # serializes one phase behind another.
            def n_att_units(b):
                return kts[b] + 1  # per (qh, s): +1 norm

            def drive(A, nA, Bunits, nB):
                Bit = iter(Bunits)
                done_b = 0
                step = 0
                for _ in A:
                    step += 1
                    want = (nB * step) // max(1, nA)
                    while done_b < want:
                        if next(Bit, None) is None:
                            done_b = nB
                            break
                        done_b += 1
                for _ in Bit:
                    pass

            import itertools

            def proj_count(b):
                ktb = kts[b]
                Kb = ktb * P
                n = 4  # Q spans
                for o in range(0, Kb, 1024):
                    w = min(1024, Kb - o)
                    n += (w + 511) // 512  # K spans
                    n += (w // P + 1) // 2  # V (yields every 2 ktiles)
                return n

            for _ in gen_proj(border[0]):
                pass
            for i in range(B):
                bcur = border[i]
                for qh in range(2):
                    A = gen_att_qh(bcur, qh)
                    nA = 2 * n_att_units(bcur)
                    Bs = []
                    nB = 0
                    if qh == 0:
                        if i > 0:
                            Bs.append(gen_oproj(border[i - 1], 1, i >= 2))
                            nB += 8
                        if i + 1 < B:
                            g = gen_proj(border[i + 1])
                            Bs.append(g)
                            nB += proj_count(border[i + 1])
                    else:
                        Bs.append(gen_oproj(bcur, 0, i >= 2))
                        nB += 8
                    drive(A, nA, itertools.chain(*Bs), nB)
            for _ in gen_oproj(border[B - 1], 1, True):
                pass

    _split_multi_waits(nc, mybir)
    return nc


def _get_program(kts: tuple):
    if kts not in _PROG_CACHE:
        _PROG_CACHE[kts] = _build_program(kts)
    return _PROG_CACHE[kts]


def kernel(**inputs) -> np.ndarray:
    import ml_dtypes
    from concourse.bass_utils import run_bass_kernel_spmd

    bf = ml_dtypes.bfloat16

    q = np.asarray(inputs["queries"], dtype=np.float32)
    k = np.asarray(inputs["keys"], dtype=np.float32)
    v = np.asarray(inputs["values"], dtype=np.float32)
    vl = np.asarray(inputs["valid_lens"]).astype(np.int64)
    Wq = np.asarray(inputs["Wq"], dtype=np.float32)
    Wk = np.asarray(inputs["Wk"], dtype=np.float32)
    Wv = np.asarray(inputs["Wv"], dtype=np.float32)
    Wo = np.asarray(inputs["Wo"], dtype=np.float32)

    kts = tuple(
        S // P if vl[b] == 0 else min(S // P, int(math.ceil(vl[b] / P)))
        for b in range(B)
    )
    KT_total = sum(kts)
    nc = _get_program(kts)

    # shared (batch-level) arrays — identical on every core
    xqt = np.ascontiguousarray(q.transpose(0, 2, 1)).astype(bf)  # [B, D, S]
    xkt = np.concatenate(
        [k[b, : kts[b] * P].T for b in range(B)], axis=1
    ).astype(bf)  # [D, SK]
    xvt = np.concatenate(
        [v[b, : kts[b] * P].T for b in range(B)], axis=1
    ).astype(bf)

    m_bias = np.empty((P, KT_total), np.float32)
    m_scale = np.empty((P, KT_total), np.float32)
    col = 0
    for b in range(B):
        vlb = int(vl[b])
        for j in range(kts[b]):
            kk = j * P + np.arange(P)
            if vlb == 0:
                m_bias[:, col] = 0.0
                m_scale[:, col] = 0.0
            else:
                m_bias[:, col] = np.where(kk < vlb, 0.0, NEG)
                m_scale[:, col] = 1.0 / math.sqrt(HD)
            col += 1

    in_maps = []
    for c in range(NCORES):
        cols = slice(c * P, (c + 1) * P)  # 2 heads = 128 dims
        in_maps.append(
            {
                "xqt": xqt,
                "xkt": xkt,
                "xvt": xvt,
                "wq": np.ascontiguousarray(Wq[:, cols]).astype(bf),
                "wk": np.ascontiguousarray(Wk[:, cols]).astype(bf),
                "wv": np.ascontiguousarray(Wv[:, cols]).astype(bf),
                "wo": np.ascontiguousarray(Wo[cols, :]).astype(bf),
                "mb": m_bias,
                "ms": m_scale,
            }
        )

    globals()["_LAST_IN_MAPS"] = in_maps
    res = run_bass_kernel_spmd(nc, in_maps, list(range(NCORES))).results

    acc = res[0]["out"].astype(np.float32)
    for c in range(1, NCORES):
        acc += res[c]["out"].astype(np.float32)
    return acc
